# revision 1
# baseline (speedup 1.0000x reference)
"""Causal self-attention kernel for Trainium2, sharded over 8 NeuronCores.

Problem: B=4, T=2048, DIM=1024, H=16 heads, head_dim=64, fp32 I/O.

Sharding: (batch, head-group) pairs -> 8 shards. Core c handles batch
b = c//2 and head group g = c%2 (8 heads each). Each core computes its
q/k/v projections for its head slice, causal flash-style attention, and
a partial o_proj against its head-slice of wo. The host sums the two
partial o_proj outputs per batch (the "all-reduce") while gathering.

Layout strategy (per core):
  - Host pre-transposes x and the weight slices so the contraction dim
    (model dim) lands on SBUF partitions, and casts them to bf16.
  - Scores are computed TRANSPOSED: sT[tk, tq] = k @ q^T, so softmax'd
    probabilities come out with tk on partitions -- exactly the layout
    the attn@v matmul needs as its moving operand (lhsT = v).
  - Softmax skips max-subtraction (scores are O(1) by construction:
    q,k ~ N(0,1), dot/8), exp runs on the scalar engine straight out of
    PSUM, and the denominator is obtained for free by augmenting v with
    a ones column.
  - Causal masking inside diagonal 128-tiles is applied by one extra
    accumulating matmul (identity x (-1e9 strictly-lower-tri mask)).
"""

import numpy as np
import ml_dtypes

import concourse.bass as bass
import concourse.bacc as bacc
import concourse.mybir as mybir
import concourse.tile as tile
from concourse.bass import ds, ts
from concourse.bass_utils import run_bass_kernel_spmd

BF16 = mybir.dt.bfloat16
F32 = mybir.dt.float32

T = 2048
D = 1024
DG = 512          # head-group width (8 heads x 64)
NH = 8            # heads per core
DH = 64
P = 128
NT = T // P       # 16 t-tiles
NKO = D // P      # 8 contraction tiles for projections
NC_CHUNK = 1024   # tq chunk width for attention
NCH = T // NC_CHUNK  # 2 chunks

_CACHED = None  # (nc, input names) -- build/trace once per process

MM_N = 512  # max moving free-dim per matmul instruction


def _mm(nc, out, lhsT, rhs, start, stop, out_off=0):
    """matmul out = lhsT.T @ rhs, sliced so no piece crosses a PSUM bank
    boundary. out_off is the column offset of `out` within its psum tile."""
    n = rhs.shape[-1]
    o = 0
    while o < n:
        w = min(n - o, MM_N - ((out_off + o) % MM_N))
        nc.tensor.matmul(
            out[:, ds(o, w)], lhsT=lhsT, rhs=rhs[:, ds(o, w)],
            start=start, stop=stop,
        )
        o += w


def _build_kernel():
    nc = bacc.Bacc("TRN2", target_bir_lowering=False, debug=False)

    xT_d = nc.dram_tensor("xT", [D, T], BF16, kind="ExternalInput").ap()
    wqT_d = nc.dram_tensor("wqT", [D, DG], BF16, kind="ExternalInput").ap()
    wkT_d = nc.dram_tensor("wkT", [D, DG], BF16, kind="ExternalInput").ap()
    wvT_d = nc.dram_tensor("wvT", [D, DG], BF16, kind="ExternalInput").ap()
    woT_d = nc.dram_tensor("woT", [DG, D], BF16, kind="ExternalInput").ap()
    y_d = nc.dram_tensor("y", [T, D], F32, kind="ExternalOutput").ap()

    with tile.TileContext(nc) as tc:
        with (
            tc.tile_pool(name="const", bufs=1) as const,
            tc.tile_pool(name="sb", bufs=1) as sb,
            tc.tile_pool(name="work", bufs=4) as work,
            tc.tile_pool(name="wnorm", bufs=2) as wnorm,
            tc.tile_pool(name="stgp", bufs=2) as stgp,
            tc.tile_pool(name="ps", bufs=2, space="PSUM") as psp,
            tc.tile_pool(name="av", bufs=2, space="PSUM") as avp,
        ):
            # ---- constants ----
            # multiplicative causal mask for diag tiles: 1 where tq >= tk
            mskb = const.tile([P, P], BF16, tag="mskb")
            nc.gpsimd.memset(mskb, 1.0)
            nc.gpsimd.affine_select(
                out=mskb, in_=mskb,
                compare_op=mybir.AluOpType.is_ge,
                fill=0.0, base=0,
                pattern=[[1, P]], channel_multiplier=-1,
            )

            # ---- persistent SBUF tensors ----
            XT = sb.tile([P, NKO, T], BF16, tag="XT")
            WQT = sb.tile([P, NKO, DG], BF16, tag="WQT")
            WKT = sb.tile([P, NKO, DG], BF16, tag="WKT")
            WVT = sb.tile([P, NKO, DG], BF16, tag="WVT")
            WOT = sb.tile([P, DG // P, D], BF16, tag="WOT")
            QT = sb.tile([P, DG // P, T], BF16, tag="QT")
            KT = sb.tile([P, DG // P, T], BF16, tag="KT")
            VA = sb.tile([P, NT, NH, DH + 1], BF16, tag="VA")
            OGT = sb.tile([P, DG // P, T], BF16, tag="OGT")

            # ---- input DMAs (chunked across queues) ----
            xr = xT_d.rearrange("(ko p) t -> p ko t", p=P)
            for k in range(NKO):
                nc.sync.dma_start(XT[:, k, :], xr[:, k, :])
            for wsb, wd in ((WQT, wqT_d), (WKT, wkT_d), (WVT, wvT_d)):
                wr = wd.rearrange("(ko p) n -> p ko n", p=P)
                for k in range(NKO):
                    nc.sync.dma_start(wsb[:, k, :], wr[:, k, :])
            wor = woT_d.rearrange("(jo p) n -> p jo n", p=P)
            for j in range(DG // P):
                nc.sync.dma_start(WOT[:, j, :], wor[:, j, :])

            # v_aug ones column
            nc.gpsimd.memset(VA[:, :, :, DH], 1.0)

            # ---- projections ----
            # qT/kT: out[dg, t] with dg on partitions (4 tiles of 128)
            for wsb, dst in ((WQT, QT), (WKT, KT)):
                for dg in range(DG // P):
                    for c in range(NCH):
                        ps = psp.tile([P, NC_CHUNK], F32, tag="s")
                        for k in range(NKO):
                            _mm(
                                nc, ps,
                                lhsT=wsb[:, k, ts(dg, P)],
                                rhs=XT[:, k, ds(c * NC_CHUNK, NC_CHUNK)],
                                start=(k == 0), stop=(k == NKO - 1),
                            )
                        nc.vector.tensor_copy(dst[:, dg, ds(c * NC_CHUNK, NC_CHUNK)], ps)
            # v: natural [t, dg] layout, written per-head into VA
            for tt in range(NT):
                ps = psp.tile([P, DG], F32, tag="s")
                for k in range(NKO):
                    nc.tensor.matmul(
                        ps,
                        lhsT=XT[:, k, ts(tt, P)],
                        rhs=WVT[:, k, :],
                        start=(k == 0), stop=(k == NKO - 1),
                    )
                nc.vector.tensor_copy(
                    VA[:, tt, :, 0:DH],
                    ps.rearrange("p (h d) -> p h d", h=NH),
                )

            # ---- attention (head pairs interleaved, per tq chunk) ----
            # Paired heads live at partitions 0-63 / 64-127 of the same
            # QT/KT p-tile, so their score matmuls use disjoint PE row
            # groups (concurrent) and the pair keeps the PE fed while the
            # scalar engine runs exp for the other head.
            def attn_scores_pair(pt, c, j):
                """scores for both heads of a pair, interleaved A/B so
                adjacent PE matmuls hit disjoint row groups (rows 0-63 vs
                64-127) and execute concurrently in the array. Returns the
                two expT tiles."""
                lo = max(c * NC_CHUNK, j * P)
                w = (c + 1) * NC_CHUNK - lo
                diag = j * P >= c * NC_CHUNK
                psA = psp.tile([P, NC_CHUNK], F32, tag="s")
                psB = psp.tile([P, NC_CHUNK], F32, tag="s")
                o = 0
                while o < w:
                    ww = min(w - o, MM_N)
                    for po, ps in ((0, psA), (DH, psB)):
                        nc.tensor.matmul(
                            ps[:, ds(o, ww)],
                            lhsT=KT[po:po + DH, pt, ts(j, P)],
                            rhs=QT[po:po + DH, pt, ds(lo + o, ww)],
                            start=True, stop=True,
                        )
                    o += ww
                ets = []
                for ps in (psA, psB):
                    et = work.tile([P, NC_CHUNK], BF16, tag="et")
                    nc.scalar.activation(
                        et[:, :w], ps[:, :w],
                        mybir.ActivationFunctionType.Exp,
                        scale=0.125,
                    )
                    if diag:
                        # zero the lower-left of the diagonal 128-block (DVE
                        # is idle; keeps the mask off the busy PE)
                        nc.vector.tensor_mul(et[:, 0:P], et[:, 0:P], mskb)
                    ets.append(et)
                return ets

            def attn_av(h, av, et, c, j):
                # AV accumulate, per psum bank: bank b of this chunk
                # ([512b, 512b+512)) has its last write at tile
                # j == 8c + 4b + 3, which carries stop=True.
                lo = max(c * NC_CHUNK, j * P)
                w = (c + 1) * NC_CHUNK - lo
                s0 = lo - c * NC_CHUNK
                for b in range(NC_CHUNK // MM_N):
                    blo, bhi = b * MM_N, (b + 1) * MM_N
                    plo, phi = max(s0, blo), min(s0 + w, bhi)
                    if plo >= phi:
                        continue
                    nc.tensor.matmul(
                        av[0:DH + 1, ds(plo, phi - plo)],
                        lhsT=VA[:, j, h, :],
                        rhs=et[:, ds(plo - s0, phi - plo)],
                        start=(j == 0),
                        stop=(j == 8 * c + 4 * b + 3),
                    )

            def attn_normalize(av, dst):
                # dst: [DH, NC_CHUNK] slice; scale av rows 0..63 by 1/row64.
                # First copy the psum accumulator to SBUF so the av slot
                # frees immediately (the PE's next chunk j=0 AV waits on it);
                # the whole divide chain then runs off the critical path.
                un = wnorm.tile([DH + 1, NC_CHUNK], F32, tag="un")
                nc.vector.tensor_copy(un, av[0:DH + 1, :])
                # 1/d as exp(-ln d) on ScalarE: d is a positive softmax
                # denominator and the product feeds a bf16 multiply, so ACT
                # table accuracy is plenty; keeps the slow DVE RECIPROCAL
                # (6.5us for a 1-partition row) off the critical path.
                rec = wnorm.tile([1, NC_CHUNK], F32, tag="rec")
                nc.scalar.activation(
                    rec, un[DH:DH + 1, :], mybir.ActivationFunctionType.Ln,
                )
                recb = wnorm.tile([1, NC_CHUNK], BF16, tag="recb")
                nc.scalar.activation(
                    recb, rec, mybir.ActivationFunctionType.Exp, scale=-1.0,
                )
                # broadcast 1/d across partitions on the (idle) GPSIMD so
                # the PE stream rolls straight into the next chunk
                bcb = wnorm.tile([DH, NC_CHUNK], BF16, tag="bcb")
                nc.gpsimd.partition_broadcast(bcb, recb)
                nc.vector.tensor_mul(dst, un[0:DH, :], bcb)

            for hp in range(NH // 2):
                hA, hB = 2 * hp, 2 * hp + 1
                stg = stgp.tile([DH, T], BF16, tag="stg")
                for c in range(NCH):
                    avA = avp.tile([P, NC_CHUNK], F32, tag="av")
                    avB = avp.tile([P, NC_CHUNK], F32, tag="av")
                    jmax = (c + 1) * NC_CHUNK // P - 1
                    for j in range(jmax + 1):
                        etA, etB = attn_scores_pair(hp, c, j)
                        attn_av(hA, avA, etA, c, j)
                        attn_av(hB, avB, etB, c, j)
                    attn_normalize(avA, OGT[0:DH, hp, ds(c * NC_CHUNK, NC_CHUNK)])
                    attn_normalize(avB, stg[:, ds(c * NC_CHUNK, NC_CHUNK)])
                # partition shift 0-63 -> 64-127 via sbuf-to-sbuf DMA
                nc.sync.dma_start(OGT[DH:P, hp, :], stg[:, :])

            # ---- o_proj partial: y[t, o] = sum_j ogT[j, t] * woT[j, o] ----
            for tt in range(NT):
                ps = psp.tile([P, D], F32, tag="s")
                for jt in range(DG // P):
                    _mm(
                        nc, ps,
                        lhsT=OGT[:, jt, ts(tt, P)],
                        rhs=WOT[:, jt, :],
                        start=(jt == 0), stop=(jt == DG // P - 1),
                    )
                ysb = wnorm.tile([P, D], F32, tag="ysb")
                nc.vector.tensor_copy(ysb, ps)
                nc.sync.dma_start(y_d[ts(tt, P), :], ysb)

    # Pin Exp and Ln to the one table set holding both (same 400-piece
    # resolution); otherwise the table-load pass alternates exp_and_others /
    # natural_log, costing a ~1.4us ACT table load per softmax normalize.
    orig = bacc.get_activation_tables
    pref = "natural_log_exp_and_others"

    def tables_ln_exp_combined(arch):
        t = orig(arch)
        if pref in t:
            for name, funcs in t.items():
                if name != pref:
                    funcs.discard(mybir.ActivationFunctionType.Exp)
                    funcs.discard(mybir.ActivationFunctionType.Ln)
        return t

    bacc.get_activation_tables = tables_ln_exp_combined
    try:
        nc.compile()
    finally:
        bacc.get_activation_tables = orig
    return nc


def _get_nc():
    global _CACHED
    if _CACHED is None:
        _CACHED = _build_kernel()
    return _CACHED


def _shard_inputs(x, wq, wk, wv, wo):
    bf = ml_dtypes.bfloat16
    in_maps = []
    for core in range(8):
        b, g = divmod(core, 2)
        gs = slice(g * DG, (g + 1) * DG)
        in_maps.append({
            "xT": np.ascontiguousarray(x[b].T).astype(bf),
            "wqT": np.ascontiguousarray(wq[gs, :].T).astype(bf),
            "wkT": np.ascontiguousarray(wk[gs, :].T).astype(bf),
            "wvT": np.ascontiguousarray(wv[gs, :].T).astype(bf),
            "woT": np.ascontiguousarray(wo[:, gs].T).astype(bf),
        })
    return in_maps


def kernel(x, wq, wk, wv, wo, _trace=False, _trace_cores=None):
    x = np.asarray(x, dtype=np.float32)
    wq = np.asarray(wq, dtype=np.float32)
    wk = np.asarray(wk, dtype=np.float32)
    wv = np.asarray(wv, dtype=np.float32)
    wo = np.asarray(wo, dtype=np.float32)

    nc = _get_nc()
    in_maps = _shard_inputs(x, wq, wk, wv, wo)
    res = run_bass_kernel_spmd(
        nc, in_maps, core_ids=list(range(8)),
        trace=_trace,
        **({"trace_cores": _trace_cores} if _trace_cores else {}),
    )
    B = x.shape[0]
    y = np.zeros((B, T, D), dtype=np.float32)
    for core in range(8):
        b = core // 2
        y[b] += res.results[core]["y"]
    if _trace:
        return y, res
    return y



# revision 3
# speedup vs baseline: 1.6572x; 1.6572x over previous
"""Causal self-attention kernel for Trainium2, sharded over 8 NeuronCores.

Problem: B=4, T=2048, DIM=1024, H=16 heads, head_dim=64, fp32 I/O.

Sharding: (batch, head-group) pairs -> 8 shards. Core c handles batch
b = c//2 and head group g = c%2 (8 heads each). Each core computes its
q/k/v projections for its head slice, causal flash-style attention, and
a partial o_proj against its head-slice of wo. The host sums the two
partial o_proj outputs per batch (the "all-reduce") while gathering.

Pipeline strategy (per core): T is processed in 4 chunks of 512. Chunk
c's attention (ACT-exp-bound) is interleaved with chunk c+1's q/k/v
projections and chunk c-1's o_proj (pure PE work) so the tensor engine
never idles long enough for the HAM clock gate to re-throttle it to
1.2 GHz (which is what capped the previous version).

Per-core layout:
  - Host pre-transposes x and the weight slices so the contraction dim
    lands on SBUF partitions, and casts to bf16.
  - Scores are computed TRANSPOSED: sT[tk, tq] = k @ q^T, so softmax'd
    probabilities come out with tk on partitions -- the layout the
    attn@v matmul needs as its moving operand (lhsT = v).
  - The two heads of a pair occupy partitions 0-63 / 64-127 of the same
    QT/KT tile; their scores land in one [128, 1024] psum tile (head A
    cols 0-511, head B cols 512-1023) so ONE scalar-engine exp covers
    both heads (halves ACT instruction count).
  - Softmax skips max-subtraction (scores are O(1) by construction),
    the denominator comes free from a ones column appended to v, and
    1/denom uses the fast DVE reciprocal instead of ACT Ln/Exp.
  - Causal masking inside diagonal 128-tiles: DVE multiply with a
    0/1 lower-triangle mask after the exp.
"""

import numpy as np
import ml_dtypes

import concourse.bass as bass
import concourse.bacc as bacc
import concourse.mybir as mybir
import concourse.tile as tile
from concourse.bass import ds, ts
from concourse.bass_utils import run_bass_kernel_spmd

BF16 = mybir.dt.bfloat16
F32 = mybir.dt.float32

T = 2048
D = 1024
DG = 512          # head-group width (8 heads x 64)
NH = 8            # heads per core
DH = 64
P = 128
NKO = D // P      # 8 contraction tiles for projections
W = 512           # tq chunk width
NCH = T // W      # 4 chunks
NTC = W // P      # 4 t-tiles per chunk
NPAIR = NH // 2   # 4 head pairs

_CACHED = None  # (nc, input names) -- build/trace once per process


def _build_kernel():
    nc = bacc.Bacc("TRN2", target_bir_lowering=False, debug=False)

    xT_d = nc.dram_tensor("xT", [D, T], BF16, kind="ExternalInput").ap()
    wqT_d = nc.dram_tensor("wqT", [D, DG], BF16, kind="ExternalInput").ap()
    wkT_d = nc.dram_tensor("wkT", [D, DG], BF16, kind="ExternalInput").ap()
    wvT_d = nc.dram_tensor("wvT", [D, DG], BF16, kind="ExternalInput").ap()
    woT_d = nc.dram_tensor("woT", [DG, D], BF16, kind="ExternalInput").ap()
    y_d = nc.dram_tensor("y", [T, D], F32, kind="ExternalOutput").ap()

    with tile.TileContext(nc) as tc:
        with (
            tc.tile_pool(name="const", bufs=1) as const,
            tc.tile_pool(name="sb", bufs=1) as sb,
            tc.tile_pool(name="work", bufs=4) as work,
            tc.tile_pool(name="wnorm", bufs=2) as wnorm,
            tc.tile_pool(name="stgp", bufs=2) as stgp,
            tc.tile_pool(name="ysbp", bufs=2) as ysbp,
            tc.tile_pool(name="ps", bufs=2, space="PSUM") as psp,
            tc.tile_pool(name="av", bufs=2, space="PSUM") as avp,
            tc.tile_pool(name="pj", bufs=2, space="PSUM") as pjp,
        ):
            # ---- constants ----
            # multiplicative causal mask for diag tiles: 1 where tq >= tk
            mskb = const.tile([P, P], BF16, tag="mskb")
            nc.gpsimd.memset(mskb, 1.0)
            nc.gpsimd.affine_select(
                out=mskb, in_=mskb,
                compare_op=mybir.AluOpType.is_ge,
                fill=0.0, base=0,
                pattern=[[1, P]], channel_multiplier=-1,
            )

            # ---- persistent SBUF tensors ----
            XT = sb.tile([P, NKO, T], BF16, tag="XT")
            WQT = sb.tile([P, NKO, DG], BF16, tag="WQT")
            WKT = sb.tile([P, NKO, DG], BF16, tag="WKT")
            WVT = sb.tile([P, NKO, DG], BF16, tag="WVT")
            WOT = sb.tile([P, DG // P, D], BF16, tag="WOT")
            QT = sb.tile([P, DG // P, T], BF16, tag="QT")
            KT = sb.tile([P, DG // P, T], BF16, tag="KT")
            VA = sb.tile([P, T // P, NH, DH + 1], BF16, tag="VA")
            OGT = sb.tile([P, DG // P, T], BF16, tag="OGT")

            # ---- input DMAs ----
            # wq + x chunk0 first so chunk-0 projections start ASAP.
            xr = xT_d.rearrange("(ko p) t -> p ko t", p=P)
            wqr = wqT_d.rearrange("(ko p) n -> p ko n", p=P)
            wkr = wkT_d.rearrange("(ko p) n -> p ko n", p=P)
            wvr = wvT_d.rearrange("(ko p) n -> p ko n", p=P)
            for k in range(NKO):
                nc.sync.dma_start(WQT[:, k, :], wqr[:, k, :])
            for k in range(NKO):
                nc.sync.dma_start(XT[:, k, 0:W], xr[:, k, 0:W])
            for k in range(NKO):
                nc.sync.dma_start(WKT[:, k, :], wkr[:, k, :])
            for k in range(NKO):
                nc.sync.dma_start(WVT[:, k, :], wvr[:, k, :])
            for k in range(NKO):
                nc.sync.dma_start(XT[:, k, W:T], xr[:, k, W:T])
            wor = woT_d.rearrange("(jo p) n -> p jo n", p=P)
            for j in range(DG // P):
                nc.sync.dma_start(WOT[:, j, :], wor[:, j, :])

            # v_aug ones column
            nc.gpsimd.memset(VA[:, :, :, DH], 1.0)

            # ---- projection / o_proj emitters (also used as PE filler) ----
            def proj_qk(wsb, dst, c, dg):
                ps = pjp.tile([P, W], F32, tag="pj")
                for k in range(NKO):
                    nc.tensor.matmul(
                        ps,
                        lhsT=wsb[:, k, ts(dg, P)],
                        rhs=XT[:, k, ds(c * W, W)],
                        start=(k == 0), stop=(k == NKO - 1),
                    )
                nc.vector.tensor_copy(dst[:, dg, ds(c * W, W)], ps)

            def proj_v(c, tl):
                tt = c * NTC + tl
                ps = pjp.tile([P, W], F32, tag="pj")
                for k in range(NKO):
                    nc.tensor.matmul(
                        ps,
                        lhsT=XT[:, k, ts(tt, P)],
                        rhs=WVT[:, k, :],
                        start=(k == 0), stop=(k == NKO - 1),
                    )
                nc.vector.tensor_copy(
                    VA[:, tt, :, 0:DH],
                    ps.rearrange("p (h d) -> p h d", h=NH),
                )

            def proj_groups(c):
                gs = []
                for dg in range(DG // P):
                    gs.append(lambda dg=dg: proj_qk(WQT, QT, c, dg))
                for dg in range(DG // P):
                    gs.append(lambda dg=dg: proj_qk(WKT, KT, c, dg))
                for tl in range(NTC):
                    gs.append(lambda tl=tl: proj_v(c, tl))
                return gs

            def oproj_tt(c, tl):
                tt = c * NTC + tl
                ysb = ysbp.tile([P, D], F32, tag="ysb")
                for piece in range(2):
                    ps = pjp.tile([P, W], F32, tag="pj")
                    for jt in range(DG // P):
                        nc.tensor.matmul(
                            ps,
                            lhsT=OGT[:, jt, ts(tt, P)],
                            rhs=WOT[:, jt, ds(piece * W, W)],
                            start=(jt == 0), stop=(jt == DG // P - 1),
                        )
                    nc.vector.tensor_copy(ysb[:, ds(piece * W, W)], ps)
                nc.sync.dma_start(y_d[ts(tt, P), :], ysb)

            def oproj_groups(c):
                return [lambda tl=tl: oproj_tt(c, tl) for tl in range(NTC)]

            # ---- attention ----
            LAG = 2  # j-iterations of score/exp lookahead before each AV

            def emit_av(pair, avA, avB, et, j, off, w, jmax):
                for h, av in ((0, avA), (1, avB)):
                    nc.tensor.matmul(
                        av[0:DH + 1, ds(off, w)],
                        lhsT=VA[:, j, 2 * pair + h, :],
                        rhs=et[:, ds(h * W, w)],
                        start=(j == 0), stop=(j == jmax),
                    )

            def normalize(av, dst):
                # copy psum out first so the av slot frees immediately
                un = wnorm.tile([DH + 1, W], F32, tag="un")
                nc.vector.tensor_copy(un, av[0:DH + 1, :])
                # the custom-DVE reciprocal is lane-locked: move the denom
                # row to partition 0 first (plain copies may shift base)
                den = wnorm.tile([1, W], F32, tag="den")
                nc.vector.tensor_copy(den, un[DH:DH + 1, :])
                rec = wnorm.tile([1, W], F32, tag="rec")
                nc.vector.reciprocal_approx_fast(rec, den)
                bcb = wnorm.tile([DH, W], F32, tag="bcb")
                nc.gpsimd.partition_broadcast(bcb, rec)
                nc.vector.tensor_mul(dst, un[0:DH, :], bcb)

            def attention_pair(pair, c, pull_filler):
                jmax = (c + 1) * NTC - 1
                avA = avp.tile([P, W], F32, tag="av")
                avB = avp.tile([P, W], F32, tag="av")
                pend = []
                for j in range(jmax + 1):
                    off = max(0, j * P - c * W)
                    w = W - off
                    lo = max(c * W, j * P)
                    diag = j * P >= c * W
                    ps = psp.tile([P, 2 * W], F32, tag="s")
                    for h in range(2):
                        nc.tensor.matmul(
                            ps[:, ds(h * W, w)],
                            lhsT=KT[h * DH:(h + 1) * DH, pair, ts(j, P)],
                            rhs=QT[h * DH:(h + 1) * DH, pair, ds(lo, w)],
                            start=True, stop=True,
                        )
                    et = work.tile([P, 2 * W], BF16, tag="et")
                    nc.scalar.activation(
                        et[:, 0:W + w], ps[:, 0:W + w],
                        mybir.ActivationFunctionType.Exp,
                        scale=0.125,
                    )
                    if diag:
                        nc.vector.tensor_mul(et[:, 0:P], et[:, 0:P], mskb)
                        nc.vector.tensor_mul(
                            et[:, ds(W, P)], et[:, ds(W, P)], mskb)
                    pend.append((j, et, off, w))
                    if len(pend) > LAG:
                        ja, eta, offa, wa = pend.pop(0)
                        emit_av(pair, avA, avB, eta, ja, offa, wa, jmax)
                    pull_filler()
                for ja, eta, offa, wa in pend:
                    emit_av(pair, avA, avB, eta, ja, offa, wa, jmax)
                normalize(avA, OGT[0:DH, pair, ds(c * W, W)])
                stg = stgp.tile([DH, W], BF16, tag="stg")
                normalize(avB, stg)
                # partition shift 0-63 -> 64-127 via sbuf-to-sbuf DMA
                nc.sync.dma_start(OGT[DH:P, pair, ds(c * W, W)], stg)

            # ---- main schedule ----
            proj0 = proj_groups(0)
            for g in proj0:
                g()

            for c in range(NCH):
                fillers = []
                if c + 1 < NCH:
                    fillers += proj_groups(c + 1)
                if c >= 1:
                    fillers += oproj_groups(c - 1)
                total_slots = NPAIR * ((c + 1) * NTC)
                state = {"slot": 0, "done": 0}

                def pull_filler():
                    state["slot"] += 1
                    want = len(fillers) * state["slot"] // total_slots
                    while state["done"] < want:
                        fillers[state["done"]]()
                        state["done"] += 1

                for pair in range(NPAIR):
                    attention_pair(pair, c, pull_filler)
                while state["done"] < len(fillers):
                    fillers[state["done"]]()
                    state["done"] += 1

            for g in oproj_groups(NCH - 1):
                g()

    nc.compile()
    return nc


def _get_nc():
    global _CACHED
    if _CACHED is None:
        _CACHED = _build_kernel()
    return _CACHED


def _shard_inputs(x, wq, wk, wv, wo):
    bf = ml_dtypes.bfloat16
    in_maps = []
    for core in range(8):
        b, g = divmod(core, 2)
        gs = slice(g * DG, (g + 1) * DG)
        in_maps.append({
            "xT": np.ascontiguousarray(x[b].T).astype(bf),
            "wqT": np.ascontiguousarray(wq[gs, :].T).astype(bf),
            "wkT": np.ascontiguousarray(wk[gs, :].T).astype(bf),
            "wvT": np.ascontiguousarray(wv[gs, :].T).astype(bf),
            "woT": np.ascontiguousarray(wo[:, gs].T).astype(bf),
        })
    return in_maps


def kernel(x, wq, wk, wv, wo, _trace=False, _trace_cores=None):
    x = np.asarray(x, dtype=np.float32)
    wq = np.asarray(wq, dtype=np.float32)
    wk = np.asarray(wk, dtype=np.float32)
    wv = np.asarray(wv, dtype=np.float32)
    wo = np.asarray(wo, dtype=np.float32)

    nc = _get_nc()
    in_maps = _shard_inputs(x, wq, wk, wv, wo)
    res = run_bass_kernel_spmd(
        nc, in_maps, core_ids=list(range(8)),
        trace=_trace,
        **({"trace_cores": _trace_cores} if _trace_cores else {}),
    )
    B = x.shape[0]
    y = np.zeros((B, T, D), dtype=np.float32)
    for core in range(8):
        b = core // 2
        y[b] += res.results[core]["y"]
    if _trace:
        return y, res
    return y
